# revision 1
# baseline (speedup 1.0000x reference)
"""Trainium2 Bass kernel for a 3rd-order HONU layer.

Math: out[b] = sum_{i<=j<=k} w3[i,j,k] * xb[b,i] * xb[b,j] * xb[b,k]
with xb = [1, x] (129 features), w3 = `weight` in lexicographic
combinations_with_replacement order (366145 entries).

Restructuring (no gathers on device):
  - Pairs (j,k), j<=k, lex order; pair index (j,k) -> Q(j) + (k-j).
  - Dense W2[129, 8385]: W2[i, p(j,k)] = w3[i,j,k] for i<=j else 0;
    contiguous block-copy from the lexicographic weight layout.
  - out[b] = sum_p (xb[b,j]*xb[b,k]) * U[b,p],  U = xb @ W2.

Sharding (combination axis across 8 cores, SPMD-uniform program):
  - j round-robin: core c, slot s in [0,17) handles j = 8s+c; slot width
    fixed at 129-8s (tail zero-padded) so the program is identical on all
    cores; per-core differences live only in the data.
  - The single i=128 weight (pair (128,128), w3[128,128,128]*x127^3) is
    added on the host, so the device contraction is exactly K=128.
  - xsh[b,t] = xb[b,t+c] (host-shifted xb) lets the device form monomial
    pairs with compile-time offsets: pair (j=8s+c, k=j+u) has
    P = xsh[:,8s] * xsh[:,8s+u].

Device pipeline per core (raw Bass, one sync-wait per instruction):
  - PE: per batch-half, 3 matmuls U = xbT.T @ W2slice into PSUM groups.
  - GPSIMD (overlapped with PE): prebuilds P columns into SBUF.
  - Vector: one fused op per PSUM group: out=(P*1)*U, accum_out=rowsum;
    then reduce -> per-batch-half column [128,1].
  - PE: transpose the two [128,1] columns via identity matmul to a
    contiguous [1,256] PSUM row (a [128,1] DMA is 128 4-byte descriptors
    and its semaphore straggle dominated the runtime).
  - Vector copies the row to SBUF; sync DMAs one contiguous [256,1] out.
Host sums the 8 per-core partials.
"""

import os

import numpy as np

import concourse.bass as bass
import concourse.mybir as mybir
from concourse.bass_utils import run_bass_kernel_spmd

# ---- problem constants (hardcoded; kernel.py must be self-contained) ----
N = 129                      # features incl. bias column
B = 256                      # batch
N_CORES = 8
NPAIR = N * (N + 1) // 2     # 8385
N_SLOTS = 17
SLOT_W = [N - 8 * s for s in range(N_SLOTS)]           # 129, 121, ..., 1
SLOT_OFF = [0]
for _w in SLOT_W:
    SLOT_OFF.append(SLOT_OFF[-1] + _w)
L = SLOT_OFF[-1]             # 1105 local columns per core
# PSUM tile groups of whole slots; widths 363, 445, 297 (all <= 512)
GROUPS = [(0, 3), (3, 8), (8, 17)]

_MM_DT_NAME = os.environ.get("HONU_MM_DT", "bfloat16")
_MM_DT = getattr(mybir.dt, _MM_DT_NAME)
_F32 = mybir.dt.float32

LAST_RESULTS = None          # BassKernelResults of the most recent run


def _np_mm_dtype():
    if _MM_DT_NAME == "bfloat16":
        import ml_dtypes
        return ml_dtypes.bfloat16
    return np.float32


def _build_bass():
    nc = bass.Bass()
    mm128_d = nc.dram_tensor("mm128", [128, B + L], _MM_DT, kind="ExternalInput")
    xsh_d = nc.dram_tensor("xsh", [B, N], _F32, kind="ExternalInput")
    idn_d = nc.dram_tensor("idn", [128, 128], _F32, kind="ExternalInput")
    out_d = nc.dram_tensor("out", [B, 1], _F32, kind="ExternalOutput")

    mult = mybir.AluOpType.mult

    from contextlib import ExitStack
    with ExitStack() as ctx:
        ec = ctx.enter_context
        mm128_t = ec(nc.sbuf_tensor("mm128_t", [128, B + L], _MM_DT))
        xsh0_t = ec(nc.sbuf_tensor("xsh0_t", [128, N], _F32))
        xsh1_t = ec(nc.sbuf_tensor("xsh1_t", [128, N], _F32))
        idn_t = ec(nc.sbuf_tensor("idn_t", [128, 128], _F32))
        scrP0_t = ec(nc.sbuf_tensor("scrP0_t", [128, L], _F32))
        scrP1_t = ec(nc.sbuf_tensor("scrP1_t", [128, L], _F32))
        scr0_t = ec(nc.sbuf_tensor("scr0_t", [128, L], _F32))
        scr1_t = ec(nc.sbuf_tensor("scr1_t", [128, L], _F32))
        acc0_t = ec(nc.sbuf_tensor("acc0_t", [128, 4], _F32))
        acc1_t = ec(nc.sbuf_tensor("acc1_t", [128, 4], _F32))
        o0_t = ec(nc.sbuf_tensor("o0_t", [128, 1], _F32))
        o1_t = ec(nc.sbuf_tensor("o1_t", [128, 1], _F32))
        orow_t = ec(nc.sbuf_tensor("orow_t", [1, B], _F32))
        deadv_t = ec(nc.sbuf_tensor("deadv_t", [1, 1], _F32))
        deadg_t = ec(nc.sbuf_tensor("deadg_t", [1, 1], _F32))
        psums = [ec(nc.psum_tensor(f"ps{i}", [128, 512], _F32))
                 for i in range(6)]
        psT = ec(nc.psum_tensor("psT", [1, 512], _F32))
        mm_sem = ec(nc.semaphore("mm_sem"))
        xsh_sem = ec(nc.semaphore("xsh_sem"))
        idn_sem = ec(nc.semaphore("idn_sem"))
        p_sem = ec(nc.semaphore("p_sem"))
        pe_sem = ec(nc.semaphore("pe_sem"))
        stt_sem = ec(nc.semaphore("stt_sem"))
        dve_sem = ec(nc.semaphore("dve_sem"))
        dma_sem = ec(nc.semaphore("dma_sem"))
        block = ec(nc.Block())
        xsh_ts = [xsh0_t, xsh1_t]
        scrP_ts = [scrP0_t, scrP1_t]
        scr_ts = [scr0_t, scr1_t]
        acc_ts = [acc0_t, acc1_t]
        o_ts = [o0_t, o1_t]

        @block.sync
        def _(sync):
            # weights split by PSUM group so the first matmuls start as
            # soon as the first chunk's completion increments land
            b1 = B + SLOT_OFF[GROUPS[0][1]]
            b2 = B + SLOT_OFF[GROUPS[1][1]]
            sync.dma_start(mm128_t[:, 0:b1], mm128_d[:, 0:b1]).then_inc(mm_sem, 16)
            sync.dma_start(mm128_t[:, b1:b2], mm128_d[:, b1:b2]).then_inc(mm_sem, 16)
            sync.dma_start(mm128_t[:, b2:B + L], mm128_d[:, b2:B + L]).then_inc(mm_sem, 16)
            sync.dma_start(xsh0_t[:], xsh_d[0:128, :]).then_inc(xsh_sem, 16)
            sync.dma_start(xsh1_t[:], xsh_d[128:256, :]).then_inc(xsh_sem, 16)
            sync.dma_start(idn_t[:], idn_d[:]).then_inc(idn_sem, 16)
            sync.wait_ge(dve_sem, 3)
            sync.dma_start(out_d[:, :], orow_t[0:1, :]).then_inc(dma_sem, 16)
            sync.wait_ge(dma_sem, 16)

        @block.tensor
        def _(tensor):
            # group-major: group g needs only weight chunk g
            for gi, (s0, s1) in enumerate(GROUPS):
                g0, g1 = SLOT_OFF[s0], SLOT_OFF[s1]
                tensor.wait_ge(mm_sem, 16 * (gi + 1))
                for bb in range(2):
                    psv = psums[bb * 3 + gi][:, :g1 - g0]
                    nc.tensor.matmul(
                        psv,
                        lhsT=mm128_t[:, bb * 128:(bb + 1) * 128],
                        rhs=mm128_t[:, B + g0:B + g1],
                        start=True, stop=True,
                    ).then_inc(pe_sem, 1)
            # observe the identity DMA once, on a dead op
            tensor.wait_ge(idn_sem, 16)
            nc.tensor.matmul(psT[0:1, 500:501], lhsT=idn_t[0:1, 0:1],
                             rhs=idn_t[0:1, 0:1], start=True, stop=True)
            # transpose [128,1] result columns to one [1,256] row:
            # psT[0, n] = o_col[n, 0] via identity rhs
            tensor.wait_ge(dve_sem, 1)
            nc.tensor.matmul(psT[0:1, 0:128], lhsT=o0_t[:, 0:1],
                             rhs=idn_t[:, :], start=True, stop=True)
            tensor.wait_ge(dve_sem, 2)
            nc.tensor.matmul(psT[0:1, 128:256], lhsT=o1_t[:, 0:1],
                             rhs=idn_t[:, :], start=True,
                             stop=True).then_inc(pe_sem, 1)

        @block.gpsimd
        def _(gpsimd):
            # xsh + identity loads on the SWDGE, overlapping the weight DMAs
            gpsimd.dma_start(xsh0_t[:], xsh_d[0:128, :]).then_inc(xsh_sem, 16)
            gpsimd.dma_start(xsh1_t[:], xsh_d[128:256, :]).then_inc(xsh_sem, 16)
            gpsimd.dma_start(idn_t[:], idn_d[:]).then_inc(idn_sem, 16)

        @block.vector
        def _(vector):
            # observe the xsh DMA queue once (dead write)
            vector.wait_ge(xsh_sem, 32)
            nc.vector.tensor_scalar_mul(
                deadv_t[0:1, 0:1], xsh0_t[0:1, 0:1], 1.0
            )
            done = 0
            for gi, (s0, s1) in enumerate(GROUPS):
                g0 = SLOT_OFF[s0]
                for bb in range(2):
                    done += 1
                    vector.wait_ge(pe_sem, done)
                    ps = psums[bb * 3 + gi]
                    for s in range(s0, s1):
                        w = SLOT_W[s]
                        lo = SLOT_OFF[s]
                        ins = nc.vector.scalar_tensor_tensor(
                            out=scr_ts[bb][:, lo:lo + w],
                            in0=xsh_ts[bb][:, 8 * s:8 * s + w],
                            scalar=xsh_ts[bb][:, 8 * s:8 * s + 1],
                            in1=ps[:, lo - g0:lo - g0 + w],
                            op0=mult,
                            op1=mult,
                            accum_out=scrP_ts[bb][:, s:s + 1],
                        )
            ins.then_inc(stt_sem, 1)
            # drain same-engine accumulator writes before the reduces
            vector.wait_ge(stt_sem, 1)
            for bb in range(2):
                nc.vector.reduce_sum(
                    o_ts[bb][:], scrP_ts[bb][:, 0:N_SLOTS],
                    axis=mybir.AxisListType.X
                ).then_inc(dve_sem, 1)
            # copy the transposed row out of PSUM (then one linear DMA)
            vector.wait_ge(pe_sem, 7)
            nc.vector.tensor_copy(
                orow_t[0:1, :], psT[0:1, 0:B]
            ).then_inc(dve_sem, 1)
    return nc


_NC_CACHE = None


def _get_nc():
    global _NC_CACHE
    if _NC_CACHE is None:
        _NC_CACHE = _build_bass()
    return _NC_CACHE


def _host_prep(x, weight):
    """Build per-core input maps from the full inputs."""
    mmdt = _np_mm_dtype()
    xb = np.concatenate([np.ones((B, 1), np.float32), x], axis=1)  # [256,129]

    # Global dense W2 [129, 8385] (rows i=0..127 used on device)
    W2 = np.zeros((N, NPAIR), np.float32)
    off = 0
    for i in range(N):
        m = (N - i) * (N - i + 1) // 2
        W2[i, NPAIR - m:] = weight[off:off + m]
        off += m

    def Q(j):
        return j * N - j * (j - 1) // 2

    xbt = np.ascontiguousarray(xb[:, :128].T)                    # [128, 256]
    idn = np.eye(128, dtype=np.float32)

    in_maps = []
    for c in range(N_CORES):
        W2L = np.zeros((128, L), np.float32)
        for s in range(N_SLOTS):
            j = 8 * s + c
            if j >= N:
                continue
            w = N - j
            W2L[:, SLOT_OFF[s]:SLOT_OFF[s] + w] = W2[:128, Q(j):Q(j) + w]
        xsh = np.zeros((B, N), np.float32)
        xsh[:, :N - c] = xb[:, c:]
        mm128 = np.concatenate([xbt, W2L], axis=1).astype(mmdt)
        in_maps.append({
            "mm128": np.ascontiguousarray(mm128),
            "xsh": xsh,
            "idn": idn,
        })
    return in_maps


def kernel(x, weight, comb_idx=None):
    """Full inputs in, full output out. comb_idx is implied by the fixed
    lexicographic layout and is not used."""
    global LAST_RESULTS
    x = np.asarray(x, dtype=np.float32)
    weight = np.asarray(weight, dtype=np.float32)
    in_maps = _host_prep(x, weight)
    nc = _get_nc()
    res = run_bass_kernel_spmd(nc, in_maps, list(range(N_CORES)))
    LAST_RESULTS = res
    out = np.zeros((B, 1), np.float64)
    for r in res.results:
        out += r["out"].astype(np.float64)
    # the single i=128 term (pair (128,128), i=j=k=128), kept off-device
    # so the device contraction is exactly K=128
    out[:, 0] += weight[-1].astype(np.float64) * x[:, 127].astype(np.float64) ** 3
    return out.astype(np.float32)



# revision 5
# speedup vs baseline: 1.1628x; 1.1628x over previous
"""Trainium2 Bass kernel for a 3rd-order HONU layer.

Math: out[b] = sum_{i<=j<=k} w3[i,j,k] * xb[b,i] * xb[b,j] * xb[b,k]
with xb = [1, x] (129 features), w3 = `weight` in lexicographic
combinations_with_replacement order (366145 entries).

Restructuring (no gathers on device):
  - Pairs (j,k), j<=k, lex order; pair index (j,k) -> Q(j) + (k-j).
  - Dense W2[129, 8385]: W2[i, p(j,k)] = w3[i,j,k] for i<=j else 0;
    contiguous block-copy from the lexicographic weight layout.
  - out[b] = sum_p (xb[b,j]*xb[b,k]) * U[b,p],  U = xb @ W2.

Sharding (combination axis across 8 cores, SPMD-uniform program):
  - j round-robin: core c, slot s in [0,17) handles j = 8s+c; slot width
    fixed at 129-8s (tail zero-padded) so the program is identical on all
    cores; per-core differences live only in the data.
  - The single i=128 weight (pair (128,128), w3[128,128,128]*x127^3) is
    added on the host, so the device contraction is exactly K=128.
  - xsh[b,t] = xb[b,t+c] (host-shifted xb) lets the device form monomial
    pairs with compile-time offsets: pair (j=8s+c, k=j+u) has
    P = xsh[:,8s] * xsh[:,8s+u].

Engine split (measured rates: vector ~1.05ns/col + ~180ns/op,
scalar ~3ns/col + ~100ns/op, gpsimd ~12.5ns/col + ~250ns/op):
  - Sync queue (HWDGE): weight chunks 0-2 (bf16, fat rows).
  - Scalar queue (HWDGE): one fp32 "xall" = [xsh0 | xsh1 | idn].
  - Scalar engine: primes the ACT table with a dummy op, then prebuilds
    P (pair products) for slots 7-13 of both halves.
  - GpSimd: prebuilds P for the tiny slots 14-16 of both halves.
  - PE: 6 matmuls U = xbT.T @ W2slice (groups of slots 0-2 / 3-6 / 7-16
    per half, widths 363/372/370, own PSUM bank each).
  - Vector: slots 0-6 as fused scalar_tensor_tensor per slot (14 ops),
    slots 7-16 as one wide STT per half over the prebuilt P; each op's
    row-sum lands in its own acc column (16 cols), then a same-engine
    barrier + dummy read drains the DVE accumulator.
  - PE transposes acc [128,16] via identity matmul to psT [16,128];
    scalar copies psT to SBUF; sync DMAs the [16,128] block out.
Host sums the per-half rows across the 8 per-core partials.
"""

import os

import numpy as np

import concourse.bass as bass
import concourse.mybir as mybir
from concourse.bass_utils import run_bass_kernel_spmd

# ---- problem constants (hardcoded; kernel.py must be self-contained) ----
N = 129                      # features incl. bias column
B = 256                      # batch
N_CORES = 8
NPAIR = N * (N + 1) // 2     # 8385
N_SLOTS = 17
SLOT_W = [N - 8 * s for s in range(N_SLOTS)]           # 129, 121, ..., 1
SLOT_OFF = [0]
for _w in SLOT_W:
    SLOT_OFF.append(SLOT_OFF[-1] + _w)
L = SLOT_OFF[-1]             # 1105 local columns per core
# PSUM tile groups of whole slots; widths 363, 372, 370 (all <= 512)
GROUPS = [(0, 3), (3, 7), (7, 17)]
G2_OFF = SLOT_OFF[7]         # 735; group-2 columns are 735..1105
G2_W = L - G2_OFF            # 370
# xall column layout (fp32): xsh half0 | xsh half1 | identity
XC_H = [0, N]                # xsh start col per half
XC_IDN = 2 * N               # 258
XALL_W = 2 * N + 128         # 386

# vector op order: (half, kind) where kind is a slot index or 'wide';
# acc/psT/orow row index = position in this list
VOPS = ([(0, s) for s in range(0, 3)] + [(1, s) for s in range(0, 3)]
        + [(0, s) for s in range(3, 7)] + [(1, s) for s in range(3, 7)]
        + [(0, "wide"), (1, "wide")])
HALF_OF_COL = [h for h, _ in VOPS]
NCOL = len(VOPS)             # 16

_MM_DT_NAME = os.environ.get("HONU_MM_DT", "bfloat16")
_MM_DT = getattr(mybir.dt, _MM_DT_NAME)
_F32 = mybir.dt.float32

LAST_RESULTS = None          # BassKernelResults of the most recent run


def _np_mm_dtype():
    if _MM_DT_NAME == "bfloat16":
        import ml_dtypes
        return ml_dtypes.bfloat16
    return np.float32


def _build_bass():
    nc = bass.Bass()
    mmw_d = nc.dram_tensor("mmw", [128, B + L], _MM_DT, kind="ExternalInput")
    xall_d = nc.dram_tensor("xall", [128, XALL_W], _F32, kind="ExternalInput")
    out_d = nc.dram_tensor("out", [NCOL, 128], _F32, kind="ExternalOutput")

    mult = mybir.AluOpType.mult

    c0 = B + SLOT_OFF[GROUPS[0][1]]     # end col of weight chunk 0
    c1 = B + SLOT_OFF[GROUPS[1][1]]     # end col of weight chunk 1

    from contextlib import ExitStack
    with ExitStack() as ctx:
        ec = ctx.enter_context
        mmw_t = ec(nc.sbuf_tensor("mmw_t", [128, B + L], _MM_DT))
        xall_t = ec(nc.sbuf_tensor("xall_t", [128, XALL_W], _F32))
        p0_t = ec(nc.sbuf_tensor("p0_t", [128, G2_W], _F32))
        p1_t = ec(nc.sbuf_tensor("p1_t", [128, G2_W], _F32))
        scr_t = ec(nc.sbuf_tensor("scr_t", [128, 384], _F32))
        acc_t = ec(nc.sbuf_tensor("acc_t", [128, NCOL], _F32))
        dead_t = ec(nc.sbuf_tensor("dead_t", [128, 1], _F32))
        orow_t = ec(nc.sbuf_tensor("orow_t", [NCOL, 128], _F32))
        psums = [ec(nc.psum_tensor(f"ps{i}", [128, 512], _F32))
                 for i in range(6)]
        psT = ec(nc.psum_tensor("psT", [NCOL, 128], _F32))
        wS0 = ec(nc.semaphore("wS0"))    # weight chunk 0 complete
        wS1 = ec(nc.semaphore("wS1"))    # weight chunk 1 complete
        wS2 = ec(nc.semaphore("wS2"))    # weight chunk 2 complete
        wA = ec(nc.semaphore("wA"))      # scalar queue: xall
        p0_sem = ec(nc.semaphore("p0_sem"))   # P half0 ready (2 producers)
        p1_sem = ec(nc.semaphore("p1_sem"))   # P half1 ready (2 producers)
        pe_sem = ec(nc.semaphore("pe_sem"))
        v_sem = ec(nc.semaphore("v_sem"))
        d_sem = ec(nc.semaphore("d_sem"))
        block = ec(nc.Block())
        p_ts = [p0_t, p1_t]
        p_sems = [p0_sem, p1_sem]

        @block.sync
        def _(sync):
            sync.dma_start(mmw_t[:, 0:c0], mmw_d[:, 0:c0]).then_inc(wS0, 16)
            sync.dma_start(mmw_t[:, c0:c1], mmw_d[:, c0:c1]).then_inc(wS1, 16)
            sync.dma_start(mmw_t[:, c1:B + L],
                           mmw_d[:, c1:B + L]).then_inc(wS2, 16)
            sync.wait_ge(v_sem, NCOL + 2)
            sync.dma_start(out_d[:, :], orow_t[:, :]).then_inc(d_sem, 16)
            sync.wait_ge(d_sem, 16)

        @block.scalar
        def _(scalar):
            # prime the ACT table (1.3us one-time load) on garbage data
            # before any real dependency
            nc.scalar.mul(dead_t[:, 0:1], dead_t[:, 0:1], 1.0)
            scalar.dma_start(xall_t[:], xall_d[:]).then_inc(wA, 16)
            scalar.wait_ge(wA, 16)
            # prebuild P slots 7-13, half0 first (vector consumes h0 first)
            for h in range(2):
                xc = XC_H[h]
                for s in range(7, 14):
                    w = SLOT_W[s]
                    lo = SLOT_OFF[s] - G2_OFF
                    ins = nc.scalar.mul(
                        p_ts[h][:, lo:lo + w],
                        xall_t[:, xc + 8 * s:xc + 8 * s + w],
                        xall_t[:, xc + 8 * s:xc + 8 * s + 1],
                    )
                ins.then_inc(p_sems[h], 1)
            # copy the transposed result rows out of PSUM
            scalar.wait_ge(pe_sem, 7)
            nc.scalar.copy(orow_t[:, :], psT[:, :]).then_inc(v_sem, 1)

        @block.gpsimd
        def _(gpsimd):
            gpsimd.wait_ge(wA, 16)
            for h in range(2):
                xc = XC_H[h]
                for s in range(14, 17):
                    w = SLOT_W[s]
                    lo = SLOT_OFF[s] - G2_OFF
                    ins = nc.gpsimd.tensor_scalar_mul(
                        p_ts[h][:, lo:lo + w],
                        xall_t[:, xc + 8 * s:xc + 8 * s + w],
                        xall_t[:, xc + 8 * s:xc + 8 * s + 1],
                    )
                ins.then_inc(p_sems[h], 1)

        @block.tensor
        def _(tensor):
            for gi, (s0, s1) in enumerate(GROUPS):
                g0c, g1c = SLOT_OFF[s0], SLOT_OFF[s1]
                tensor.wait_ge([wS0, wS1, wS2][gi], 16)
                for h in range(2):
                    nc.tensor.matmul(
                        psums[2 * gi + h][:, :g1c - g0c],
                        lhsT=mmw_t[:, h * 128:(h + 1) * 128],
                        rhs=mmw_t[:, B + g0c:B + g1c],
                        start=True, stop=True,
                    ).then_inc(pe_sem, 1)
            # transpose acc [128,16] -> psT [16,128] via identity rhs
            tensor.wait_ge(v_sem, NCOL + 1)
            nc.tensor.matmul(psT[:, :], lhsT=acc_t[:, :],
                             rhs=xall_t[:, XC_IDN:XC_IDN + 128],
                             start=True, stop=True).then_inc(pe_sem, 1)

        @block.vector
        def _(vector):
            vector.wait_ge(wA, 16)
            for col, (h, kind) in enumerate(VOPS):
                xc = XC_H[h]
                if kind == "wide":
                    vector.wait_ge(pe_sem, 5 + h)
                    vector.wait_ge(p_sems[h], 2)
                    nc.vector.scalar_tensor_tensor(
                        out=scr_t[:, :G2_W],
                        in0=p_ts[h][:, :],
                        scalar=1.0,
                        in1=psums[4 + h][:, :G2_W],
                        op0=mult, op1=mult,
                        accum_out=acc_t[:, col:col + 1],
                    ).then_inc(v_sem, 1)
                else:
                    s = kind
                    gi = 0 if s < 3 else 1
                    g0c = SLOT_OFF[GROUPS[gi][0]]
                    w = SLOT_W[s]
                    lo = SLOT_OFF[s]
                    vector.wait_ge(pe_sem, 2 * gi + h + 1)
                    nc.vector.scalar_tensor_tensor(
                        out=scr_t[:, :w],
                        in0=xall_t[:, xc + 8 * s:xc + 8 * s + w],
                        scalar=xall_t[:, xc + 8 * s:xc + 8 * s + 1],
                        in1=psums[2 * gi + h][:, lo - g0c:lo - g0c + w],
                        op0=mult, op1=mult,
                        accum_out=acc_t[:, col:col + 1],
                    ).then_inc(v_sem, 1)
            # same-engine barrier + dependent read drains the DVE
            # accumulator before the PE transpose consumes acc
            vector.wait_ge(v_sem, NCOL)
            nc.vector.tensor_copy(
                dead_t[:, 0:1], acc_t[:, NCOL - 1:NCOL]
            ).then_inc(v_sem, 1)
    return nc


_NC_CACHE = None


def _get_nc():
    global _NC_CACHE
    if _NC_CACHE is None:
        _NC_CACHE = _build_bass()
    return _NC_CACHE


def _host_prep(x, weight):
    """Build per-core input maps from the full inputs."""
    mmdt = _np_mm_dtype()
    xb = np.concatenate([np.ones((B, 1), np.float32), x], axis=1)  # [256,129]

    # Global dense W2 [129, 8385] (rows i=0..127 used on device)
    W2 = np.zeros((N, NPAIR), np.float32)
    off = 0
    for i in range(N):
        m = (N - i) * (N - i + 1) // 2
        W2[i, NPAIR - m:] = weight[off:off + m]
        off += m

    def Q(j):
        return j * N - j * (j - 1) // 2

    xbt = np.ascontiguousarray(xb[:, :128].T)                    # [128, 256]
    idn = np.eye(128, dtype=np.float32)

    in_maps = []
    for c in range(N_CORES):
        W2L = np.zeros((128, L), np.float32)
        for s in range(N_SLOTS):
            j = 8 * s + c
            if j >= N:
                continue
            w = N - j
            W2L[:, SLOT_OFF[s]:SLOT_OFF[s] + w] = W2[:128, Q(j):Q(j) + w]
        xsh = np.zeros((B, N), np.float32)
        xsh[:, :N - c] = xb[:, c:]
        mmw = np.concatenate([xbt, W2L], axis=1).astype(mmdt)
        xall = np.concatenate(
            [xsh[0:128, :], xsh[128:256, :], idn], axis=1)       # [128, 386]
        in_maps.append({
            "mmw": np.ascontiguousarray(mmw),
            "xall": np.ascontiguousarray(xall),
        })
    return in_maps


def kernel(x, weight, comb_idx=None):
    """Full inputs in, full output out. comb_idx is implied by the fixed
    lexicographic layout and is not used."""
    global LAST_RESULTS
    x = np.asarray(x, dtype=np.float32)
    weight = np.asarray(weight, dtype=np.float32)
    in_maps = _host_prep(x, weight)
    nc = _get_nc()
    res = run_bass_kernel_spmd(nc, in_maps, list(range(N_CORES)))
    LAST_RESULTS = res
    out = np.zeros((B,), np.float64)
    for r in res.results:
        o = r["out"].astype(np.float64)          # [16, 128]
        for col, h in enumerate(HALF_OF_COL):
            out[h * 128:(h + 1) * 128] += o[col]
    # the single i=128 term (pair (128,128), i=j=k=128), kept off-device
    # so the device contraction is exactly K=128
    out += weight[-1].astype(np.float64) * x[:, 127].astype(np.float64) ** 3
    return out.astype(np.float32)[:, None]
